# revision 1
# baseline (speedup 1.0000x reference)
"""IntersectionLoss Trainium2 kernel.

Math: loss_n = maskedmean_j relu(R + S*log(sum_i exp(-|t2_nj - t1_ni|^2/S) * m1_i + eps))
Key identity: |t2_j - t1_i|^2 = n2_j + n1_i - 2*t2_j.t1_i, so the full exponent
  x_ji = 2*t2_j.t1_i/S - n1_i/S + ln m1_i - n2_j/S
is a K=5 bilinear form: with augmented rows (gamma^2 = 2/(S*128))
  t1aug = [gamma*x, gamma*y, gamma*z, (-n1_i/S + ln m1_i)/128, 1]
  t2aug = [gamma*x, gamma*y, gamma*z, 1,            1 - n2_j/(S*128)]
one PE matmul produces t_ji = 1 + x_ji/128 directly in PSUM.

The L1-reduction of exp(x) then runs on TWO engines in parallel (each plane's
two 1024-wide PSUM chunks go to different engines):
  - ACT chunks: scalar.activation Exp with scale=128, bias=-128 (exp(128t-128)
    == exp(x)) and the sum riding accum_out — 1 elem/lane/cycle at 1.2 GHz.
  - GPSIMD chunks: tensor_scalar pow(t, 128)*1 with accum_out computes
    t^128 = (1+x/128)^128 ~ exp(x) plus the row sum in one ucode instruction
    (vpowf on the 8 Q7 cores).
The (1+x/128)^128 approximation under-counts acc by <~1% in the mass-carrying
terms; the resulting loss error is ~1e-3 relative (tolerance 2e-2).

Sharding: data-parallel over N=16 across 8 cores (2 batches per core). Final
log/relu/masked-mean over the (N,2048) accumulator runs on host in float64.
"""

import sys

sys.path.insert(0, "/opt/trn_rl_repo")

import numpy as np

import concourse.bass as bass
import concourse.tile as tile
from concourse import mybir
from concourse.bass_utils import run_bass_kernel_spmd

RADIUS = 1.0
SIGMA = 2.5
EPSILON = 1e-12

N, L1, L2 = 16, 2048, 2048
NCORES = 8
NB = N // NCORES  # batches per core
P = 128
A = L2 // P  # 16 j-tiles per batch
F32 = mybir.dt.float32
BF16 = mybir.dt.bfloat16
F32R = mybir.dt.float32r
AF = mybir.ActivationFunctionType

_CACHE = {}

H = 2  # chunks per plane: ACT eats half 0, GPSIMD half 1
CH = L1 // H  # 1024 elems = 2 PSUM banks; 4 rotating buffers = 8 banks
# ACT sustains ~1184ns/chunk, the DVE-copy+GPSIMD-pow lane ~1550ns, so ACT
# additionally takes the GPSIMD half of 6 planes (38/26 split); planes 30/31
# are stolen so the slower pow lane is not the last to finish (sim-swept).
ACT_STEAL = frozenset({5, 11, 17, 23, 30, 31})


def _build_program():
    nc = bass.Bass()
    # taug[b, k, s, i]: s=0 -> t1aug row k, s=1 -> t2aug row k (i in 0..2047)
    taug_d = nc.declare_dram_parameter("taug", (NB, 5, 2, L1), F32R, isOutput=False)
    acc_a_d = nc.declare_dram_parameter("acc_a", (P, NB * A * H), F32, isOutput=True)
    acc_v_d = nc.declare_dram_parameter("acc_v", (P, NB * A * H), F32, isOutput=True)
    NMM = CH // 512  # matmuls per chunk

    with tile.TileContext(nc) as tc:
        with (
            tc.tile_pool(name="consts", bufs=1) as consts,
            tc.tile_pool(name="sb", bufs=2) as sb,
            # one staging tile per GPSIMD chunk (never reused): the DVE copy
            # then waits only on its PE fill — a rotating pool would add a
            # WAR wait and overflow the 1-wait queue structs
            tc.tile_pool(name="stage", bufs=NB * A - len(ACT_STEAL)) as stage,
            tc.tile_pool(name="ps", bufs=4, space="PSUM") as ps,
        ):
            # broadcast exponent operand for the GPSIMD pow tensor_tensor
            c128 = consts.tile([P, 1], F32)
            nc.gpsimd.memset(c128[:], 128.0)
            # ACT bias const (exp(128t - 128)); memset is tile-tracked so the
            # first activation gets a proper Pool->ACT dependency
            bias_t = consts.tile([P, 1], F32)
            nc.gpsimd.memset(bias_t[:], -128.0)

            # single input DMA: one completion semaphore, so every matmul
            # carries at most one sync wait (the PE Matmult queue struct
            # fits only one; see _elide_redundant_matmul_waits)
            tT = consts.tile([5, NB * 2 * L1], F32R)
            nc.sync.dma_start(
                out=tT.rearrange("k (b s i) -> k b s i", b=NB, s=2),
                in_=taug_d.rearrange("b k s i -> k b s i"),
            )

            # PE warm-up: tiny matmuls start the pstate ramp (full clock needs
            # ~3us of continuous busy) before the real fills arrive. The
            # operand is an UNTRACKED SBUF alloc read uninitialized — no
            # dependency, so the ramp starts right after init; the outputs
            # are garbage in ring slots that real fills overwrite.
            warm_ap = nc.alloc_sbuf_tensor("warm_fodder", [5, 192], F32R).ap()
            for w in range(8):
                gw = ps.tile([P, CH], F32, tag="ps")
                nc.tensor.matmul(
                    gw[:, :64], warm_ap[:, :128], warm_ap[:, 128:192],
                    start=True, stop=True,
                )

            # separate per-engine accumulators so ACT and DVE never touch the
            # same tile; one column per chunk, merged on host. Zeroed on
            # device: each engine writes only its own columns and the host
            # sums both tensors — the DMA'd-out bytes of the other engine's
            # columns must be 0.0, not uninitialized SBUF.
            acc_act = sb.tile([P, NB * A * H], F32, tag="acc_act")
            acc_dve = sb.tile([P, NB * A * H], F32, tag="acc_dve")
            # acc_act memset LAST: the table-warm exp below waits on it, and
            # that single Pool-sem wait then also covers the acc_dve memset
            nc.gpsimd.memset(acc_dve[:], 0.0)
            nc.gpsimd.memset(acc_act[:], 0.0)
            # warm the Exp table while the input DMA is in flight (real HW
            # charges ~1.3us for the first table load); its accum lands in
            # acc_act[:, 0] — overwritten by the real chunk — which also
            # absorbs the acc-memset wait onto the ACT queue, keeping the
            # first real Activation at one sync wait
            tiny = consts.tile([P, 1], F32)
            nc.scalar.activation(
                tiny[:],
                bias_t[:],
                AF.Exp,
                bias=bias_t[:],
                scale=0.0,
                accum_out=acc_act[:, 0:1],
            )
            pending_reduce = []  # deferred (dump, col) so DVE copies never
            # queue behind a reduce that is still waiting on the GPSIMD pow

            def flush_reduce():
                for dump, col in pending_reduce:
                    nc.vector.tensor_scalar(
                        dump[:],
                        dump[:],
                        1.0,
                        0.0,
                        mybir.AluOpType.mult,
                        mybir.AluOpType.add,
                        accum_out=acc_dve[:, col : col + 1],
                    )
                pending_reduce.clear()

            for b in range(NB):
                for jt in range(A):
                    lhsT = tT[:, (2 * b + 1) * L1 + jt * P : (2 * b + 1) * L1 + (jt + 1) * P]
                    plane = b * A + jt
                    for h in range(H):
                        g = ps.tile([P, CH], F32, tag="ps")
                        for it in range(NMM):
                            i0 = 2 * b * L1 + h * CH + it * 512
                            nc.tensor.matmul(
                                g[:, it * 512 : (it + 1) * 512],
                                lhsT,
                                tT[:, i0 : i0 + 512],
                                start=True,
                                stop=True,
                            )
                        col = plane * H + h
                        if h == 0 or plane in ACT_STEAL:
                            nc.scalar.activation(
                                g[:],
                                g[:],
                                AF.Exp,
                                bias=bias_t[:],
                                scale=128.0,
                                accum_out=acc_act[:, col : col + 1],
                            )
                        else:
                            # GPSIMD cannot read PSUM: DVE stages the chunk to
                            # SBUF (f32 — pow amplifies rounding x128, bf16
                            # staging would cost ~25% accuracy), then GPSIMD
                            # computes pow(t,128)*1 with the row-sum riding
                            # accum_out. Tiles rotate (3 bufs) so the copy,
                            # the pow, and the next copy pipeline.
                            scr = stage.tile([P, CH], F32, tag="scr")
                            nc.vector.tensor_scalar(
                                scr[:], g[:], 1.0, None, mybir.AluOpType.mult
                            )
                            dump = stage.tile([P, CH], BF16, tag="dump")
                            nc.gpsimd.tensor_tensor(
                                dump[:],
                                scr[:],
                                c128[:].to_broadcast((P, CH)),
                                mybir.AluOpType.pow,
                            )
                            flush_reduce()
                            pending_reduce.append((dump, col))
                    if plane == NB * A // 2 - 1:
                        # drain the first batch's accumulators while the second
                        # batch computes; only the tail columns ride the final DMA
                        flush_reduce()
                        half = A * H
                        nc.sync.dma_start(
                            out=acc_a_d[:, :half], in_=acc_act[:, :half]
                        )
                        nc.sync.dma_start(
                            out=acc_v_d[:, :half], in_=acc_dve[:, :half]
                        )
            flush_reduce()
            half = A * H
            # acc_v first: the pow lane finishes ~0.7us before ACT, so its
            # descriptor processes on the serial HWDGE queue while the last
            # ACT chunks are still running
            nc.sync.dma_start(out=acc_v_d[:, half:], in_=acc_dve[:, half:])
            nc.sync.dma_start(out=acc_a_d[:, half:], in_=acc_act[:, half:])

    _elide_redundant_matmul_waits(nc)
    return nc


def _elide_redundant_matmul_waits(nc):
    """Drop semaphore waits on Matmult instrs that are transitively implied by
    their other waits (Tile emits per-proc-minimal, not transitively-minimal,
    waits; the PE Matmult queue struct only fits one sync wait command).

    Soundness: a wait (S, v) is removed only if chaining (a) same-engine
    in-order start/completion and (b) the completion vector clocks of the
    producers of the REMAINING waits already guarantees S >= v.
    """

    def merge(dst, src):
        for k, v in src.items():
            if dst.get(k, 0) < v:
                dst[k] = v

    all_insts = []
    for bb in nc.bb_map.values():
        all_insts.extend(bb.bb.instructions)
    if True:
        insts = all_insts
        n = len(insts)
        # cumulative updater ticks per semaphore
        sem_updaters = {}  # sem -> list of (cum_value, idx)
        sem_cum = {}
        idx_updates = [[] for _ in range(n)]  # idx -> [(sem, cum_after)]
        for idx, inst in enumerate(insts):
            si = inst.sync_info
            if not si:
                continue
            for u in si.on_update:
                s = u.ant_name
                v = getattr(u, "update_value", None) or 1
                c = sem_cum.get(s, 0) + v
                sem_cum[s] = c
                sem_updaters.setdefault(s, []).append((c, idx))
                idx_updates[idx].append((s, c))

        def producer_of(s, v):
            for c, uidx in sem_updaters.get(s, ()):
                if c >= v:
                    return uidx
            return None

        start_clock = [dict() for _ in range(n)]
        comp_clock = [dict() for _ in range(n)]
        for _ in range(3):
            prev_start = {}
            prev_comp = {}
            for idx, inst in enumerate(insts):
                e = str(inst.engine)
                sc = dict(prev_start.get(e, {}))
                si = inst.sync_info
                if si:
                    for w in si.on_wait:
                        s, v = w.ant_name, w.wait_value
                        if sc.get(s, 0) < v:
                            sc[s] = v
                        p = producer_of(s, v)
                        if p is not None:
                            merge(sc, comp_clock[p])
                cc = dict(sc)
                merge(cc, prev_comp.get(e, {}))
                for s, c in idx_updates[idx]:
                    if cc.get(s, 0) < c:
                        cc[s] = c
                start_clock[idx] = sc
                comp_clock[idx] = cc
                prev_start[e] = sc
                prev_comp[e] = cc

        # drop same-engine waits on multi-wait instructions: each engine
        # executes its queue in order, so a wait whose updaters are all
        # earlier instructions of the same engine is redundant (most queue
        # structs only fit one sync wait)
        for idx, inst in enumerate(insts):
            si = inst.sync_info
            if not si or len(si.on_wait) <= 1:
                continue
            eng = str(inst.engine)
            kept = []
            for w in si.on_wait:
                need = [
                    uidx
                    for c, uidx in sem_updaters.get(w.ant_name, ())
                    if 1 <= c <= w.wait_value
                ]
                if need and all(
                    uidx < idx and str(insts[uidx].engine) == eng for uidx in need
                ):
                    continue  # implied by same-engine program order
                kept.append(w)
            if not kept:
                kept = [si.on_wait[-1]]
            if len(kept) < len(si.on_wait):
                si.on_wait = kept
                inst.sync_info = si

        # elide waits implied by remaining waits + engine order
        prev_start = {}
        for idx, inst in enumerate(insts):
            e = str(inst.engine)
            si = inst.sync_info
            if si and len(si.on_wait) > 1:
                waits = list(si.on_wait)
                kept = list(waits)
                for w in waits:
                    if len(kept) <= 1:
                        break
                    others = [x for x in kept if x is not w]
                    implied = dict(prev_start.get(e, {}))
                    for o in others:
                        if implied.get(o.ant_name, 0) < o.wait_value:
                            implied[o.ant_name] = o.wait_value
                        p = producer_of(o.ant_name, o.wait_value)
                        if p is not None:
                            merge(implied, comp_clock[p])
                    if implied.get(w.ant_name, 0) >= w.wait_value:
                        kept = others
                if len(kept) < len(waits):
                    si.on_wait = kept
                    inst.sync_info = si
            sc = dict(prev_start.get(e, {}))
            if si:
                for w in si.on_wait:
                    if sc.get(w.ant_name, 0) < w.wait_value:
                        sc[w.ant_name] = w.wait_value
                    p = producer_of(w.ant_name, w.wait_value)
                    if p is not None:
                        merge(sc, comp_clock[p])
            prev_start[e] = sc


def _prep(t1, t2, mask1):
    """Build taug (N,5,2,L1) on host; the matmul then yields t = 1 + x/128."""
    n1 = np.einsum("nik,nik->ni", t1, t1)  # (N, L1)
    n2 = np.einsum("njk,njk->nj", t2, t2)  # (N, L2)
    with np.errstate(divide="ignore"):
        # clamp so masked-out (m1=0) entries give |t| < 1 -> t^128 ~ 0
        lnm1 = np.maximum(np.log(mask1), -120.0)
    gamma = np.sqrt(2.0 / (SIGMA * 128.0)).astype(np.float32)
    taug = np.empty((N, 5, 2, L1), np.float32)
    taug[:, 0:3, 0, :] = t1.transpose(0, 2, 1) * gamma
    taug[:, 3, 0, :] = (-n1 / SIGMA + lnm1) / 128.0
    taug[:, 4, 0, :] = 1.0
    taug[:, 0:3, 1, :] = t2.transpose(0, 2, 1) * gamma
    taug[:, 3, 1, :] = 1.0
    taug[:, 4, 1, :] = 1.0 - n2 / (SIGMA * 128.0)
    return taug


def _make_in_maps(t1, t2, mask1, mask2):
    t1 = np.asarray(t1, dtype=np.float32)
    t2 = np.asarray(t2, dtype=np.float32)
    mask1 = np.asarray(mask1, dtype=np.float32)
    taug = _prep(t1, t2, mask1)
    return [{"taug": taug[c * NB : (c + 1) * NB]} for c in range(NCORES)]


def kernel(t1, t2, mask1, mask2):
    if "nc" not in _CACHE:
        _CACHE["nc"] = _build_program()
    nc = _CACHE["nc"]

    in_maps = _make_in_maps(t1, t2, mask1, mask2)
    res = run_bass_kernel_spmd(nc, in_maps, list(range(NCORES)))

    # per core: acc[p, (b*A+jt)*H + h], j = jt*128+p; each column written by
    # exactly one engine (the other output stays zero), so summing the two
    # outputs and then the H halves merges everything
    acc = np.stack(
        [
            (r["acc_a"] + r["acc_v"]).reshape(P, NB, A, H).sum(axis=-1)
            for r in res.results
        ]
    )  # (C,P,NB,A)
    acc_full = acc.transpose(0, 2, 3, 1).reshape(N, L2).astype(np.float64)

    d = RADIUS + SIGMA * np.log(acc_full + EPSILON)
    d = np.maximum(d, 0.0)
    m2 = np.asarray(mask2).astype(np.float64)
    loss = (d * m2).sum(axis=-1) / m2.sum(axis=-1)
    return loss.astype(np.float32)



# revision 7
# speedup vs baseline: 2.5712x; 2.5712x over previous
"""IntersectionLoss Trainium2 kernel — Mehler eigen-expansion.

Math: loss_n = maskedmean_j relu(R + S*log(sum_i exp(-|t2_nj - t1_ni|^2/S) * m1_i + eps))

Instead of evaluating the (L2,L1) pairwise exp directly (exp-throughput
bound at ~45us/core), expand the Gaussian kernel in its Mehler/eigen
basis. For any rho in (0,1), per coordinate:

  e^{-eps^2 (x-y)^2} = sqrt(1-rho^2) sum_n h_n(cx)h_n(cy) e^{-s x^2} e^{-s y^2}
     h_n(z) = H_n(z) sqrt(rho^n/(2^n n!)),  c^2 = eps^2(1-rho^2)/rho,
     s = eps^2(1-rho),  eps^2 = 1/SIGMA.

In 3D the eigenvalues decay like rho^(a+b+c); truncating at total degree
K=6 (D=84 features) gives loss rel err ~7e-5 on these inputs (tolerance
2e-2). The i-reduction collapses to V_D = sum_i u_i F1[i,D] (one tiny PE
matmul chain) and acc_j = env2_j * F2[j,:] . V — no pairwise work at all.

Device pipeline per core (2 batches, both sides, all fp16 on DVE at the
2x 16-bit rate; feature/pair layouts keep a packed innermost dim):
  DMA in z=c*x (fp16) + u = m1*env1 ->
  Pool: per-step prescales zsA_n = z*A_n (t-scaled so each DVE recurrence
        step is two plain tensor_tensors: tmp = zsA.h'_n; h'_{n+1} = tmp - h'_{n-1})
  DVE:  Hermite recurrence -> degree-ordered pair pyramid PAB=Hx*Hy ->
        feature pyramid F = PAB * Hz (per c-block, broadcast) ->
  PE:   V_b[1,84] = sum_chunk u^T F1 (PSUM accum), broadcast matmul
        ones[1,128] x Vs -> VB[128,168]
  DVE:  P = F2 * VB (one op, both batches), grouped tensor_reduce ->
        raw[128,(ch,b)] -> DMA out.
Host: fold side-1 envelope into u; apply side-2 envelope + prefactor in
log space on the (N,L2) accumulator (fp64), then relu + masked mean —
same O(N*L) host pre/post work as the direct-kernel baseline.
"""

import sys

sys.path.insert(0, "/opt/trn_rl_repo")

import numpy as np

import concourse.bass as bass
import concourse.tile as tile
from concourse import mybir
from concourse.bass_utils import run_bass_kernel_spmd

RADIUS = 1.0
SIGMA = 2.5
EPSILON = 1e-12
EPS2 = 1.0 / SIGMA

N, L1, L2 = 16, 2048, 2048
NCORES = 8
NB = N // NCORES  # batches per core
P = 128
NCH = L1 // P  # 16 point-chunks per batch side

K = 6  # max total feature degree
RHO = 0.28
NDEG = K + 1  # 7 hermite orders per dim

F32 = mybir.dt.float32
F16 = mybir.dt.float16
ALU = mybir.AluOpType
AX = mybir.AxisListType

# ---- feature index tables (shared by host prep and program build) ----
# pairs (a,b), a+b<=K, degree-major, a descending within a degree: the
# degree-d block is Hx[n'=K-d..K of the reversed copy] * Hy[n=0..d].
PAIRS = [(d - k, k) for d in range(NDEG) for k in range(d + 1)]
T = [((m + 1) * (m + 2)) // 2 for m in range(NDEG)]  # #pairs with a+b<=m
NP_ = T[K]  # 28
# features (c,(a,b)): c-major blocks; block c = pair-prefix of length T[K-c]
FEATS = [(c, ab) for c in range(NDEG) for ab in PAIRS[: T[K - c]]]
D = len(FEATS)  # 84

# recurrence constants: h_{n+1} = alpha_n z h_n - beta_n h_{n-1}; stored
# t-scaled h'_n = t_n h_n with t_{n+1} = t_{n-1}/beta_n so the update is
# h'_{n+1} = (z*A_n) h'_n - h'_{n-1}.
_BETA = {n: RHO * np.sqrt(n / (n + 1)) for n in range(1, K)}
_ALPHA = {n: np.sqrt(2 * RHO / (n + 1)) for n in range(1, K)}
_TS = [1.0, 1.0]
for n in range(1, K):
    _TS.append(_TS[n - 1] / _BETA[n])
_A = {n: _TS[n + 1] * _ALPHA[n] / _TS[n] for n in range(1, K)}
_WSQ = np.array(
    [1.0 / (_TS[a] * _TS[b] * _TS[c]) ** 2 for (c, (a, b)) in FEATS], np.float32
)

_CACHE = {}

# free-axis layouts (innermost stride 1 = q or b so 16-bit DVE ops hit 2x)
NQ = 2 * NB  # 4 (side, batch) tiles; q = 2*side + batch
ZCOLS = NCH * 3 * NQ  # z block (ch, d, q)
UCOLS = NCH * NB  # u block (ch, b)


def _zoff(ch, d, q):
    return ch * (3 * NQ) + d * NQ + q


def _hoff(n, ch, d, q):
    return n * ZCOLS + ch * (3 * NQ) + d * NQ + q


def _build_program():
    nc = bass.Bass()
    zu_d = nc.declare_dram_parameter("zu", (P, ZCOLS + UCOLS), F16, isOutput=False)
    cst_d = nc.declare_dram_parameter("cst", (1, D * NB), F32, isOutput=False)
    raw_d = nc.declare_dram_parameter("raw", (P, NCH * NB), F32, isOutput=True)

    with tile.TileContext(nc) as tc:
        with (
            tc.tile_pool(name="sb", bufs=1) as sb,
            tc.tile_pool(name="ps", bufs=1, space="PSUM") as ps,
        ):
            zu = sb.tile([P, ZCOLS + UCOLS], F16, tag="zu")
            wsq = sb.tile([1, D * NB], F32, tag="wsq")
            nc.sync.dma_start(out=zu[:], in_=zu_d[:])
            nc.sync.dma_start(out=wsq[:], in_=cst_d[:])

            Z = zu[:, :ZCOLS].rearrange("p (c d q) -> p c d q", c=NCH, d=3)
            U = zu[:, ZCOLS:].rearrange("p (c b) -> p c b", c=NCH)

            ones = sb.tile([1, P], F16, tag="ones")
            nc.gpsimd.memset(ones[:], 1.0)

            # wait absorbers: several engine queue structs fit only ONE sync
            # wait command, so give each engine an early op that waits on the
            # input DMAs / memsets; later real ops then carry a single wait
            # (the rest are same-engine-implied and elided below).
            scratch = sb.tile([1, 2], F16, tag="scratch")
            nc.vector.tensor_copy(scratch[:], wsq[:, :2])  # DVE absorbs cst
            jps = ps.tile([1, 1], F32, tag="jps")
            nc.tensor.matmul(  # PE absorbs zu DMA
                jps[:], zu[:, :1], zu[:, :1], start=True, stop=True
            )
            jps2 = ps.tile([P, 1], F32, tag="jps2")
            nc.tensor.matmul(  # PE absorbs ones memset
                jps2[:], ones[:], ones[:, :1], start=True, stop=True
            )

            # Hermite values, t-scaled: H[n, ch, d, q]
            H = sb.tile([P, NDEG * ZCOLS], F16, tag="H")
            Hv = H[:].rearrange("p (n c d q) -> p n c d q", n=NDEG, c=NCH, d=3)
            nc.gpsimd.memset(H[:, :ZCOLS], 1.0)  # h'_0 = 1
            # h'_1 = z*sqrt(2 rho) (Pool, so DVE starts at step n=1)
            nc.gpsimd.tensor_scalar(
                H[:, ZCOLS : 2 * ZCOLS], zu[:, :ZCOLS], float(np.sqrt(2 * RHO)),
                None, ALU.mult,
            )
            # per-step prescaled z on Pool (overlaps the DVE recurrence)
            zsA = sb.tile([P, (K - 1) * ZCOLS], F16, tag="zsA")
            for n in range(1, K):
                nc.gpsimd.tensor_scalar(
                    zsA[:, (n - 1) * ZCOLS : n * ZCOLS], zu[:, :ZCOLS],
                    float(_A[n]), None, ALU.mult,
                )

            # DVE recurrence: 2 plain tensor_tensors per step (fp16 2x rate)
            tmp = sb.tile([P, (K - 1) * ZCOLS], F16, tag="tmp")
            for n in range(1, K):
                tn = tmp[:, (n - 1) * ZCOLS : n * ZCOLS]
                nc.vector.tensor_tensor(
                    tn, zsA[:, (n - 1) * ZCOLS : n * ZCOLS],
                    H[:, n * ZCOLS : (n + 1) * ZCOLS], ALU.mult,
                )
                nc.vector.tensor_tensor(
                    H[:, (n + 1) * ZCOLS : (n + 2) * ZCOLS], tn,
                    H[:, (n - 1) * ZCOLS : n * ZCOLS], ALU.subtract,
                )

            # reversed x-dim copies on ACT (pipelines with the recurrence):
            # HxR[n', ch, q] = H[K-n', ch, d=0, q]
            HxR = sb.tile([P, NDEG * NCH * NQ], F16, tag="HxR")
            HxRv = HxR[:].rearrange("p (n c q) -> p n c q", n=NDEG, c=NCH)
            for nr in range(NDEG - 1, -1, -1):
                nc.scalar.copy(HxRv[:, nr], Hv[:, K - nr, :, 0])

            # pair pyramid PAB[ch, pair, q] = Hx[a]*Hy[b], degree-major
            PAB = sb.tile([P, NCH * NP_ * NQ], F16, tag="PAB")
            PABv = PAB[:].rearrange("p (c r q) -> p c r q", c=NCH, r=NP_)
            for d in range(NDEG):
                lo = T[d - 1] if d else 0
                nc.vector.tensor_tensor(
                    PABv[:, :, lo : T[d]],
                    HxRv[:, K - d : K + 1].rearrange("p n c q -> p c n q"),
                    Hv[:, : d + 1, :, 1].rearrange("p n c q -> p c n q"),
                    ALU.mult,
                )

            # feature pyramid F[ch, feat, q] = PAB[prefix] * Hz[c] (bcast),
            # side-1 (q 0:2) first so PE can start its V accumulation early
            F = sb.tile([P, NCH * D * NQ], F16, tag="F")
            Fv = F[:].rearrange("p (c f q) -> p c f q", c=NCH, f=D)
            BOFF = np.concatenate([[0], np.cumsum([T[K - c] for c in range(NDEG)])])
            for qlo, qhi in ((0, NB), (NB, NQ)):
                for c in range(NDEG):
                    blen = T[K - c]
                    hz = Hv[:, c, :, 2, qlo:qhi].rearrange(
                        "p c (r q) -> p c r q", r=1
                    ).to_broadcast((P, NCH, blen, qhi - qlo))
                    nc.vector.tensor_tensor(
                        Fv[:, :, BOFF[c] : BOFF[c] + blen, qlo:qhi],
                        PABv[:, :, :blen, qlo:qhi],
                        hz,
                        ALU.mult,
                    )
                if qlo == 0:
                    # side-1 done: V_b = sum_ch u_ch^T F1_ch on PE
                    vps = [
                        ps.tile([1, D], F32, tag=f"v{b}", name=f"vps{b}")
                        for b in range(NB)
                    ]
                    for b in range(NB):
                        for ch in range(NCH):
                            nc.tensor.matmul(
                                vps[b][:],
                                U[:, ch, b : b + 1],
                                Fv[:, ch, :, b],
                                start=(ch == 0),
                                stop=(ch == NCH - 1),
                            )

            # Vs[feat*2+b] = V_b[feat] * wsq (fused t-scale correction)
            Vs = sb.tile([1, D * NB], F16, tag="Vs")
            Vsv = Vs[:].rearrange("p (f b) -> p f b", f=D)
            wv = wsq[:].rearrange("p (f b) -> p f b", f=D)
            for b in range(NB):
                nc.vector.tensor_tensor(
                    Vsv[:, :, b], vps[b][:], wv[:, :, b], ALU.mult
                )
            # broadcast V to all partitions via ones-matmul
            vbps = ps.tile([P, D * NB], F32, tag="vb")
            nc.tensor.matmul(vbps[:], ones[:], Vs[:], start=True, stop=True)
            VB = sb.tile([P, D * NB], F16, tag="VB")
            nc.vector.tensor_copy(VB[:], vbps[:])

            # P = F2 * VB for both batches in one op (innermost b packed)
            Pp = sb.tile([P, NCH * D * NB], F16, tag="P")
            Ppv = Pp[:].rearrange("p (c f b) -> p c f b", c=NCH, f=D)
            vbb = VB[:].rearrange("p (r f b) -> p r f b", r=1, f=D).to_broadcast(
                (P, NCH, D, NB)
            )
            nc.vector.tensor_tensor(Ppv[:], Fv[:, :, :, NB:NQ], vbb, ALU.mult)

            # grouped reduce over feat -> raw[ch, b]
            raw = sb.tile([P, NCH * NB], F32, tag="raw")
            nc.vector.tensor_reduce(
                raw[:].rearrange("p (c b) -> p c b", c=NCH),
                Ppv[:].rearrange("p c f b -> p c b f"),
                AX.X,
                ALU.add,
            )
            nc.sync.dma_start(out=raw_d[:], in_=raw[:])

    _elide_redundant_waits(nc)
    return nc


def _elide_redundant_waits(nc):
    """Drop semaphore waits that are transitively implied by an instruction's
    other waits (Tile emits per-proc-minimal, not transitively-minimal, waits;
    several engine queue structs only fit 1-2 sync wait commands).

    Soundness: a wait (S, v) is removed only if chaining (a) same-engine
    in-order start/completion and (b) the completion vector clocks of the
    producers of the REMAINING waits already guarantees S >= v.
    """

    def merge(dst, src):
        for k, v in src.items():
            if dst.get(k, 0) < v:
                dst[k] = v

    all_insts = []
    for bb in nc.bb_map.values():
        all_insts.extend(bb.bb.instructions)
    insts = all_insts
    n = len(insts)
    sem_updaters = {}  # sem -> list of (cum_value, idx)
    sem_cum = {}
    idx_updates = [[] for _ in range(n)]
    for idx, inst in enumerate(insts):
        si = inst.sync_info
        if not si:
            continue
        for u in si.on_update:
            s = u.ant_name
            v = getattr(u, "update_value", None) or 1
            c = sem_cum.get(s, 0) + v
            sem_cum[s] = c
            sem_updaters.setdefault(s, []).append((c, idx))
            idx_updates[idx].append((s, c))

    def producer_of(s, v):
        for c, uidx in sem_updaters.get(s, ()):
            if c >= v:
                return uidx
        return None

    start_clock = [dict() for _ in range(n)]
    comp_clock = [dict() for _ in range(n)]
    for _ in range(3):
        prev_start = {}
        prev_comp = {}
        for idx, inst in enumerate(insts):
            e = str(inst.engine)
            sc = dict(prev_start.get(e, {}))
            si = inst.sync_info
            if si:
                for w in si.on_wait:
                    s, v = w.ant_name, w.wait_value
                    if sc.get(s, 0) < v:
                        sc[s] = v
                    p = producer_of(s, v)
                    if p is not None:
                        merge(sc, comp_clock[p])
            cc = dict(sc)
            merge(cc, prev_comp.get(e, {}))
            for s, c in idx_updates[idx]:
                if cc.get(s, 0) < c:
                    cc[s] = c
            start_clock[idx] = sc
            comp_clock[idx] = cc
            prev_start[e] = sc
            prev_comp[e] = cc

    # drop same-engine waits on multi-wait instructions: each engine executes
    # its queue in order, so a wait whose updaters are all earlier
    # instructions of the same engine is redundant
    for idx, inst in enumerate(insts):
        si = inst.sync_info
        if not si or len(si.on_wait) <= 1:
            continue
        eng = str(inst.engine)
        kept = []
        for w in si.on_wait:
            need = [
                uidx
                for c, uidx in sem_updaters.get(w.ant_name, ())
                if 1 <= c <= w.wait_value
            ]
            if need and all(
                uidx < idx and str(insts[uidx].engine) == eng for uidx in need
            ):
                continue
            kept.append(w)
        if not kept:
            kept = [si.on_wait[-1]]
        if len(kept) < len(si.on_wait):
            si.on_wait = kept
            inst.sync_info = si

    # elide waits implied by remaining waits + engine order
    prev_start = {}
    for idx, inst in enumerate(insts):
        e = str(inst.engine)
        si = inst.sync_info
        if si and len(si.on_wait) > 1:
            waits = list(si.on_wait)
            kept = list(waits)
            for w in waits:
                if len(kept) <= 1:
                    break
                others = [x for x in kept if x is not w]
                implied = dict(prev_start.get(e, {}))
                for o in others:
                    if implied.get(o.ant_name, 0) < o.wait_value:
                        implied[o.ant_name] = o.wait_value
                    p = producer_of(o.ant_name, o.wait_value)
                    if p is not None:
                        merge(implied, comp_clock[p])
                if implied.get(w.ant_name, 0) >= w.wait_value:
                    kept = others
            if len(kept) < len(waits):
                si.on_wait = kept
                inst.sync_info = si
        sc = dict(prev_start.get(e, {}))
        if si:
            for w in si.on_wait:
                if sc.get(w.ant_name, 0) < w.wait_value:
                    sc[w.ant_name] = w.wait_value
                p = producer_of(w.ant_name, w.wait_value)
                if p is not None:
                    merge(sc, comp_clock[p])
        prev_start[e] = sc


def _prep(t1, t2, mask1):
    """Per-core inputs: zu [P, ZCOLS+UCOLS] fp16 and the wsq constant row."""
    c_sc = np.sqrt(EPS2 * (1 - RHO**2) / RHO)
    s_env = EPS2 * (1 - RHO)
    t1 = t1.astype(np.float64)
    t2 = t2.astype(np.float64)
    env1 = np.exp(-s_env * (t1**2).sum(-1))  # (N, L1)
    u_full = (mask1.astype(np.float64) * env1).astype(np.float16)  # (N, L1)
    z1 = (c_sc * t1).astype(np.float16)  # (N, L1, 3)
    z2 = (c_sc * t2).astype(np.float16)
    cst = np.repeat(_WSQ, NB)[None, :].astype(np.float32)  # [1, D*NB]

    in_maps = []
    for cc in range(NCORES):
        zu = np.zeros((P, ZCOLS + UCOLS), np.float16)
        for b in range(NB):
            n = cc * NB + b
            for s, z in ((0, z1), (1, z2)):
                q = 2 * s + b
                # zu[p, zoff(ch,d,q)] = z[n, ch*128+p, d]
                zc = z[n].reshape(NCH, P, 3).transpose(1, 0, 2)  # (P, ch, d)
                cols = np.arange(NCH)[:, None] * (3 * NQ) + np.arange(3)[None, :] * NQ + q
                zu[:, cols.reshape(-1)] = zc.reshape(P, -1)
            uc = u_full[n].reshape(NCH, P).T  # (P, ch)
            zu[:, ZCOLS + np.arange(NCH) * NB + b] = uc
        in_maps.append({"zu": zu, "cst": cst})
    return in_maps


def kernel(t1, t2, mask1, mask2):
    if "nc" not in _CACHE:
        _CACHE["nc"] = _build_program()
    nc = _CACHE["nc"]

    t1 = np.asarray(t1, dtype=np.float32)
    t2 = np.asarray(t2, dtype=np.float32)
    mask1 = np.asarray(mask1, dtype=np.float32)
    mask2 = np.asarray(mask2, dtype=np.float32)

    in_maps = _prep(t1, t2, mask1)
    res = run_bass_kernel_spmd(nc, in_maps, list(range(NCORES)))

    # raw[p, ch*NB + b] -> acc[n, j], j = ch*128 + p
    s_env = EPS2 * (1 - RHO)
    lnpref = 1.5 * np.log1p(-(RHO**2))
    acc = np.empty((N, L2), np.float64)
    for cc in range(NCORES):
        r = res.results[cc]["raw"]  # (P, NCH*NB)
        for b in range(NB):
            n = cc * NB + b
            raw_n = r[:, np.arange(NCH) * NB + b].T.reshape(-1)  # j-major
            n2 = (t2[n].astype(np.float64) ** 2).sum(-1)
            acc[n] = np.exp(lnpref - s_env * n2 + np.log(np.maximum(raw_n, 1e-30)))

    d = RADIUS + SIGMA * np.log(acc + EPSILON)
    d = np.maximum(d, 0.0)
    m2 = mask2.astype(np.float64)
    loss = (d * m2).sum(axis=-1) / m2.sum(axis=-1)
    return loss.astype(np.float32)
